# revision 19
# baseline (speedup 1.0000x reference)
"""Distributed attention kernel for Trainium2 (8 NeuronCores).

Problem: softmax(Q @ K.T / sqrt(S)) @ V with S=8192, D=256, fp32 I/O.
Note the reference scales by sqrt(K.shape[-2]) = sqrt(S), NOT sqrt(D).

Sharding: Q rows split across 8 cores (1024 rows each); K, V replicated.
No collectives needed - each core computes its output rows independently.

Per-core algorithm:
  - DMA-load Q, K, V with inline f32->bf16 cast (SWDGE).
  - PE-transpose Q and K into Q^T [D, 1024] and K^T [D, S] (the score
    matmul contracts over D, so both operands need D on partitions); the
    PSUM->SBUF copies cast them to fp8e4m3, laid out [128, 2, n] for
    DoubleRow's (partition, k-tile) contraction.
  - Scores are computed TRANSPOSED in one fp8 DoubleRow matmul per key
    block (contraction 256 at 2 MACs/cell/cycle): S^T[keys, q] = K @ Q^T.
    The exp output P^T = exp(S^T * scale) (bf16, in SBUF) is directly the
    stationary operand (lhsT) of the P @ V matmul - no P transpose needed.
  - Scores are ~N(0, 1/32) so softmax needs no max subtraction; fp8
    score error (~0.7% on softmax weights) dominates the output rel_err
    of ~7e-3, well under the 2e-2 gate.
  - V (bf16) gets a ones-column appended: P @ [V | 1] accumulates the
    unnormalized output and the softmax denominator in one PSUM chain.
  - Normalize with a per-partition reciprocal multiply, DMA out fp32.
  - Schedule ("stag3"): chunk-0's score stream runs with its first TWO
    PV chains lagging 2-3 key-block-pairs behind the exps; chunk-1's
    score stream then interleaves with chunk-0's remaining chains and
    chunk-1's own first chain. At most 2 PV chains are live at once
    (single-PSUM-bank accumulation chains are HW-measured much faster
    than multi-chain bank-cycling patterns). Filling the exp-paced
    windows with PV matmuls this way cut ~25%+ on hardware versus the
    phase-sequential schedule, invisible to the cost model.
"""

import numpy as np

S = 8192
D = 256
N_CORES = 8
SHARD = S // N_CORES  # 1024 query rows per core

import os as _os

DEFAULT_VARIANT = _os.environ.get("KVAR", "stag3")

_CACHE = {}


def _build_vb(repeat=1, lag=3, order="jh", batch=2, nwarm=12):
    """V-stationary rewrite ("vb"):

    - Q/K transposed by the DMA XBAR (dma_start_transpose), not the PE.
    - P@V uses V blocks as the stationary operand and pt (=exp scores,
      [keys, queries]) as the moving operand: 4 psum chains (2 d-chunks x
      2 q-halves) accumulate out^T over all 64 key blocks. 256 weight
      loads instead of 512, and no ones-column.
    - softmax denominator: DVE sums pt tiles via a bf16 pair/quad/oct
      tree (4x DVE mode) + one f32 accumulate per 8 kb into
      acc[128, 1024]; at the end 8 tiny matmuls (acc tile stationary,
      ones column moving) give denom transposed [128q, 8], reciprocal on
      DVE, applied post-transpose as a per-partition ACT scale.
    - output: psum out^T -> bf16 sbuf -> XBAR transpose -> scale ->
      casting DMA store (bf16 -> f32).
    """
    import concourse.mybir as mybir
    import concourse.tile as tile
    from concourse import bacc

    f32 = mybir.dt.float32
    bf16 = mybir.dt.bfloat16
    f8 = mybir.dt.float8e4
    AF = mybir.ActivationFunctionType

    SCALE = 1.0 / float(np.sqrt(np.float32(S)))
    NKB = S // 128  # 64 key blocks
    KG = 8          # max key blocks per staged K group
    NKG = None      # group list built in _emit_vb
    KPF = 3         # K groups prefetched ahead (beyond the current)
    VPF = 2         # V slabs (8 kb each) prefetched ahead

    nc = bacc.Bacc()
    q_ext = nc.dram_tensor("Q", [SHARD, D], f32, kind="ExternalInput")
    k_ext = nc.dram_tensor("K", [S, D], f32, kind="ExternalInput")
    v_ext = nc.dram_tensor("V", [S, D], f32, kind="ExternalInput")
    out_ext = nc.dram_tensor("out", [SHARD, D], f32, kind="ExternalOutput")

    with tile.TileContext(nc) as tc:
        with (
            tc.tile_pool(name="singles", bufs=1) as singles,
            tc.tile_pool(name="kstage", bufs=4) as kstage,
            tc.tile_pool(name="kbst", bufs=4) as kbst,
            tc.tile_pool(name="ptp", bufs=lag + 5) as ptp,
            tc.tile_pool(name="trees", bufs=1) as trees,
            tc.tile_pool(name="outp", bufs=1) as outp,
            tc.tile_pool(name="stp", bufs=2, space="PSUM") as stp,
            tc.tile_pool(name="pvp", bufs=1, space="PSUM") as pvp,
        ):
            warm = singles.tile([128, 128], bf16, tag="warm", name="warm")
            nc.vector.memset(warm[:], 0.0)
            ones_col = singles.tile([128, 1], f32, tag="ones", name="ones")
            nc.vector.memset(ones_col[:], 1.0)

            for _rep in range(repeat):
                _emit_vb(nc, tc, singles, kstage, kbst, ptp, trees, outp,
                         stp, pvp, warm, ones_col, q_ext, k_ext, v_ext,
                         out_ext, mybir, SCALE, NKB, KG, NKG, KPF, VPF,
                         lag, order, batch, nwarm)

    nc.finalize()
    return nc


def _emit_vb(nc, tc, singles, kstage, kbst, ptp, trees, outp, stp, pvp,
             warm, ones_col, q_ext, k_ext, v_ext, out_ext,
             mybir, SCALE, NKB, KG, NKG, KPF, VPF, lag, order, batch, nwarm):
    f32 = mybir.dt.float32
    bf16 = mybir.dt.bfloat16
    f8 = mybir.dt.float8e4
    AF = mybir.ActivationFunctionType

    acc = singles.tile([128, SHARD], f32, tag="acc", name="acc")
    nc.vector.memset(acc[:], 0.0)

    # ---- warm-up matmuls: engage the PE clock while input DMAs land ----
    if nwarm:
        wp = stp.tile([128, 1024], f32, tag="st", name="wp")
        for _ in range(nwarm):
            nc.tensor.matmul(wp[:, 0:128], warm[:], warm[:],
                             start=True, stop=True)

    # ---- Q: casting load + one batched XBAR transpose + fp8 cast ----
    # dma_start_transpose with out [128, X, 128] (3D) transposes each
    # 128-col slab x of the 2D input independently:
    #   out[a, x, y] = in[y, x*128 + a]
    # With in = qs [128q, (n d) = n*256 + c*128 + a] the slab index is
    # x = n*2 + c, so out = qtb [128, n, c, 128] (c INNER) gives
    # qtb[a, n, c, y] = Q[n*128 + y, c*128 + a] — the transposed layout.
    kt8 = singles.tile([128, 2, S], f8, tag="kt8", name="kt8")
    k_all = k_ext.rearrange("(b p) d -> p b d", p=128)
    vt = singles.tile([128, NKB, D], bf16, tag="vt", name="vt")
    v_re = v_ext.rearrange("(b p) d -> p b d", p=128)

    def emit_k_group(b0, nb):
        ks = kstage.tile([128, KG, D], bf16, tag="ks", name="ks")
        nc.gpsimd.dma_start(out=ks[:, 0:nb, :], in_=k_all[:, b0:b0 + nb, :])
        ktb = kbst.tile([128, KG, 2, 128], bf16, tag="ktb", name="ktb")
        nc.sync.dma_start_transpose(ktb[:, 0:nb, :, :], ks[:, 0:nb, :])
        for c in range(2):
            nc.vector.tensor_copy(
                kt8[:, c, b0 * 128:(b0 + nb) * 128], ktb[:, 0:nb, c, :]
            )

    def emit_v_slab(s):
        nc.gpsimd.dma_start(
            out=vt[:, s * 8:(s + 1) * 8, :], in_=v_re[:, s * 8:(s + 1) * 8, :]
        )

    # group list: first two groups small so ST(0) starts ASAP
    kgroups = [(0, 4), (4, 4)] + [(8 * t, 8) for t in range(1, 8)]
    kg_start = {b0: gi for gi, (b0, _nb) in enumerate(kgroups)}
    kg_next = 0

    def emit_k_groups_until(gi):
        nonlocal kg_next
        while kg_next <= gi and kg_next < len(kgroups):
            emit_k_group(*kgroups[kg_next])
            kg_next += 1

    # K group 0 first: its dma->xbar->cast chain gates ST(0).
    emit_k_groups_until(0)

    # ---- Q: casting load + one batched XBAR transpose + fp8 cast ----
    qs = singles.tile([128, SHARD // 128, D], bf16, tag="qs", name="qs")
    q_re = q_ext.rearrange("(n p) d -> p n d", p=128)
    nc.gpsimd.dma_start(out=qs[:], in_=q_re[:])
    qtb = singles.tile([128, SHARD // 128, 2, 128], bf16, tag="qtb",
                       name="qtb")
    nc.sync.dma_start_transpose(qtb[:], qs[:])
    qt8 = singles.tile([128, 2, SHARD], f8, tag="qt8", name="qt8")
    for c in range(2):
        nc.vector.tensor_copy(qt8[:, c, :], qtb[:, :, c, :])

    emit_k_groups_until(KPF - 1)
    for s in range(VPF):
        emit_v_slab(s)

    # ---- main stream ----
    pts = {}
    pairs, quads = [], []
    ot = [[None, None], [None, None]]  # [c][qh] psum chains

    def st_step(kb):
        st = stp.tile([128, 1024], f32, tag="st", name="st")
        for qh in range(2):
            nc.tensor.matmul(
                st[:, qh * 512:(qh + 1) * 512],
                kt8[:, :, kb * 128:(kb + 1) * 128],
                qt8[:, :, qh * 512:(qh + 1) * 512],
                start=True,
                stop=True,
                perf_mode=mybir.MatmulPerfMode.DoubleRow,
            )
        pt = ptp.tile([128, 1024], bf16, tag="pt", name="pt")
        nc.scalar.activation(pt[:], st[:], AF.Exp, scale=SCALE)
        pts[kb] = pt

    def pv_one(c, qh, j):
        if j == 0:
            ot[c][qh] = pvp.tile([128, 512], f32, tag=f"ot{c}{qh}",
                                 name=f"ot{c}{qh}")
        nc.tensor.matmul(
            ot[c][qh][:],
            vt[:, j, c * 128:(c + 1) * 128],
            pts[j][:, qh * 512:(qh + 1) * 512],
            start=(j == 0),
            stop=(j == NKB - 1),
        )

    def pv_emit(js):
        if order == "jh":
            for c in range(2):
                for j in js:
                    for qh in range(2):
                        pv_one(c, qh, j)
        else:  # "hj"
            for c in range(2):
                for qh in range(2):
                    for j in js:
                        pv_one(c, qh, j)

    pv_next = 0
    for kb in range(NKB):
        if kb in kg_start:
            emit_k_groups_until(kg_start[kb] + KPF)
        if kb % 8 == 0:
            s = kb // 8 + VPF
            if s < NKB // 8:
                emit_v_slab(s)
        st_step(kb)
        # bf16 sum tree (DVE 4x mode) -> one f32 accumulate per 8 kb;
        # bf16 rounding on <=8-term partials adds ~6e-4 denominator error.
        if kb % 2 == 1:
            pr = trees.tile([128, 1024], bf16, tag=f"pr{(kb // 2) % 2}",
                            name="pr")
            nc.vector.tensor_tensor(pr[:], pts[kb - 1][:], pts[kb][:],
                                    mybir.AluOpType.add)
            pairs.append(pr)
        if kb % 4 == 3:
            qd = trees.tile([128, 1024], bf16, tag=f"qd{(kb // 4) % 2}",
                            name="qd")
            nc.vector.tensor_tensor(qd[:], pairs[-2][:], pairs[-1][:],
                                    mybir.AluOpType.add)
            quads.append(qd)
        if kb % 8 == 7:
            oc = trees.tile([128, 1024], bf16, tag="oc", name="oc")
            nc.vector.tensor_tensor(oc[:], quads[-2][:], quads[-1][:],
                                    mybir.AluOpType.add)
            nc.vector.tensor_tensor(acc[:], acc[:], oc[:],
                                    mybir.AluOpType.add)
        ready = kb - lag
        while pv_next <= ready - (batch - 1):
            pv_emit(list(range(pv_next, pv_next + batch)))
            pv_next += batch
    while pv_next < NKB:
        n = min(batch, NKB - pv_next)
        pv_emit(list(range(pv_next, pv_next + n)))
        pv_next += n

    # ---- tail: denominator, reciprocal, cast, transpose, scale, store ----
    dt = stp.tile([128, 1024], f32, tag="st", name="dt")
    for t in range(8):
        nc.tensor.matmul(
            dt[:, t:t + 1],
            acc[:, t * 128:(t + 1) * 128],
            ones_col[:],
            start=True,
            stop=True,
        )
    rcpT = outp.tile([128, 8], f32, tag="rcp", name="rcp")
    nc.vector.reciprocal(rcpT[:], dt[:, 0:8])

    ot_bf = outp.tile([128, 2, 1024], bf16, tag="otb", name="otb")
    for qh in range(2):
        nc.vector.tensor_copy(ot_bf[:, 0, qh * 512:(qh + 1) * 512],
                              ot[0][qh][:])
        nc.scalar.copy(ot_bf[:, 1, qh * 512:(qh + 1) * 512],
                       ot[1][qh][:])
    o = outp.tile([128, 8, D], bf16, tag="o", name="o")
    for c in range(2):
        nc.sync.dma_start_transpose(
            o[:, :, c * 128:(c + 1) * 128], ot_bf[:, c, :]
        )
    o2 = outp.tile([128, 8, D], bf16, tag="o2", name="o2")
    for t in range(8):
        nc.scalar.mul(o2[:, t, :], o[:, t, :], rcpT[:, t:t + 1])
        nc.gpsimd.dma_start(
            out=out_ext[t * 128:(t + 1) * 128, :], in_=o2[:, t, :]
        )


def _build(repeat=1, variant="stag3"):
    import concourse.mybir as mybir
    import concourse.tile as tile
    from concourse import bacc
    from concourse.masks import make_identity

    f32 = mybir.dt.float32
    bf16 = mybir.dt.bfloat16
    f8 = mybir.dt.float8e4
    AF = mybir.ActivationFunctionType

    SCALE = 1.0 / float(np.sqrt(np.float32(S)))

    NKB = S // 128      # 64 key blocks
    NQC = SHARD // 512  # 2 query chunks per core
    NQT = 4             # 128-row query tiles per chunk

    nc = bacc.Bacc()
    q_ext = nc.dram_tensor("Q", [SHARD, D], f32, kind="ExternalInput")
    k_ext = nc.dram_tensor("K", [S, D], f32, kind="ExternalInput")
    v_ext = nc.dram_tensor("V", [S, D], f32, kind="ExternalInput")
    out_ext = nc.dram_tensor("out", [SHARD, D], f32, kind="ExternalOutput")

    with tile.TileContext(nc) as tc:
        with (
            tc.tile_pool(name="singles", bufs=1) as singles,
            tc.tile_pool(
                name="kstage", bufs=5 if variant.endswith("k") else 3
            ) as kstage,
            tc.tile_pool(name="ptp", bufs=1) as ptp,
            tc.tile_pool(name="outp", bufs=4) as outp,
            tc.tile_pool(
                name="stp", bufs=3 if variant == "stag5" else 2, space="PSUM"
            ) as stp,
            tc.tile_pool(
                name="pvp",
                bufs=4 if variant == "ilv" else 2,
                space="PSUM",
            ) as pvp,
            tc.tile_pool(name="trp", bufs=2, space="PSUM") as trp,
        ):
            ident = singles.tile([128, 128], bf16, tag="ident", name="ident")
            make_identity(nc, ident)
            if variant.endswith("k"):
                variant = variant[:-1]
            if variant.endswith("w"):
                # Warm-up matmuls: fill the lead-in while the first input
                # DMAs land, and engage the HAM clock-gate (transposes
                # alone don't) so early real matmuls run at full rate.
                wp = trp.tile([128, 128], f32, tag="tr", name="wp")
                for _ in range(40):
                    nc.tensor.matmul(wp[:], ident[:], ident[:],
                                     start=True, stop=True)
                variant = variant[:-1]
            for _rep in range(repeat):
                _emit_body(nc, tc, singles, kstage, ptp, outp, stp, pvp,
                           trp, ident, q_ext, k_ext, v_ext, out_ext,
                           mybir, SCALE, NKB, NQC, NQT, variant)

    nc.finalize()
    return nc


def _emit_body(nc, tc, singles, kstage, ptp, outp, stp, pvp,
               trp, ident, q_ext, k_ext, v_ext, out_ext,
               mybir, SCALE, NKB, NQC, NQT, variant="stag3"):
    f32 = mybir.dt.float32
    bf16 = mybir.dt.bfloat16
    f8 = mybir.dt.float8e4
    AF = mybir.ActivationFunctionType

    if True:
        if True:
            # ---- Q: load (cast to fp8) + PE-transpose into QT [128,2,SHARD]
            # QT[p, c, q] = Q[q, c*128+p]; dim1 = d-chunk for DoubleRow's
            # (p, t) contraction layout.
            qs = singles.tile([128, SHARD // 128, D], bf16, tag="qs", name="qs")
            q_re = q_ext.rearrange("(n p) d -> p n d", p=128)
            nc.gpsimd.dma_start(out=qs[:, 0:4, :], in_=q_re[:, 0:4, :])
            nc.gpsimd.dma_start(out=qs[:, 4:8, :], in_=q_re[:, 4:8, :])
            st_dt = bf16 if variant == "seqbf" else f8
            qt8 = singles.tile([128, 2, SHARD], st_dt, tag="qt8", name="qt8")
            for c in range(2):
                for g in range(SHARD // 512):
                    tr = (trp.tile([128, 512], bf16, tag="tr", name="tr")
                          if variant in ("seq", "seqp", "stag", "stag2", "stag3", "stag4") else
                          stp.tile([128, 512], bf16, tag="st", name="tr"))
                    for j in range(4):
                        qi = g * 4 + j
                        nc.tensor.transpose(
                            tr[:, j * 128:(j + 1) * 128],
                            qs[:, qi, c * 128:(c + 1) * 128],
                            ident,
                        )
                    nc.vector.tensor_copy(
                        qt8[:, c, g * 512:(g + 1) * 512], tr[:]
                    )

            # ---- K: load (cast to fp8) + PE-transpose into KT [128,2,S] ----
            kt8 = singles.tile([128, 2, S], st_dt, tag="kt8", name="kt8")
            k_all = k_ext.rearrange("(b p) d -> p b d", p=128)
            groups = [(0, 4), (4, 4)] + [(8 * t, 8) for t in range(1, 8)]
            for b_start, nb in groups:
                ks = kstage.tile([128, 8, D], bf16, tag="ks", name="ks")
                nc.gpsimd.dma_start(
                    out=ks[:, 0:nb, :],
                    in_=k_all[:, b_start:b_start + nb, :],
                )
                for c in range(2):
                    for g in range(nb // 4):
                        tr = (trp.tile([128, 512], bf16, tag="tr", name="tr")
                          if variant in ("seq", "seqp", "stag", "stag2", "stag3", "stag4") else
                          stp.tile([128, 512], bf16, tag="st", name="tr"))
                        for j in range(4):
                            n = g * 4 + j
                            nc.tensor.transpose(
                                tr[:, j * 128:(j + 1) * 128],
                                ks[:, n, c * 128:(c + 1) * 128],
                                ident,
                            )
                        b0 = b_start + g * 4
                        nc.vector.tensor_copy(
                            kt8[:, c, b0 * 128:(b0 + 4) * 128], tr[:]
                        )

            # ---- V: load (cast) with a ones-column appended ----
            vo = []
            v_re = v_ext.rearrange("(t n p) d -> t p n d", p=128, n=4)
            for t in range(S // 512):
                vt = singles.tile([128, 4, D + 1], bf16, tag=f"vo{t}", name=f"vo{t}")
                nc.vector.memset(vt[:, :, D:D + 1], 1.0)
                nc.gpsimd.dma_start(out=vt[:, :, 0:D], in_=v_re[t])
                vo.append(vt)

            # ---- main: ST stream interleaved with kb-major PV chains ----
            # PV for key block kb is emitted LAG key-block-pairs after its
            # exp, so the PE always has PV matmuls to run between ST
            # matmuls instead of stalling at the exp production rate.
            pts = {0: [], 1: []}
            chains = {}

            def st_step(qc, kbp):
                st = stp.tile([128, 1024], f32, tag="st", name="st")
                ptag = (f"pt{qc}_{kbp}" if variant.startswith(("seqp", "stag"))
                        else f"pt{kbp}")
                pt = ptp.tile([128, 1024], bf16, tag=ptag, name=ptag)
                for half in range(2):
                    kb = kbp * 2 + half
                    if variant == "seqbf":
                        for c in range(2):
                            nc.tensor.matmul(
                                st[:, half * 512:(half + 1) * 512],
                                kt8[:, c, kb * 128:(kb + 1) * 128],
                                qt8[:, c, qc * 512:(qc + 1) * 512],
                                start=(c == 0),
                                stop=(c == 1),
                            )
                    else:
                        nc.tensor.matmul(
                            st[:, half * 512:(half + 1) * 512],
                            kt8[:, :, kb * 128:(kb + 1) * 128],
                            qt8[:, :, qc * 512:(qc + 1) * 512],
                            start=True,
                            stop=True,
                            perf_mode=mybir.MatmulPerfMode.DoubleRow,
                        )
                nc.scalar.activation(pt[:], st[:], AF.Exp, scale=SCALE)
                pts[qc].append(pt)

            def pv_kb(qc, kb):
                for qt_i in range(NQT):
                    if kb == 0:
                        chains[(qc, qt_i)] = pvp.tile(
                            [128, D + 1], f32, tag="pv", name="pv"
                        )
                    pv = chains[(qc, qt_i)]
                    col0 = (kb % 2) * 512 + qt_i * 128
                    nc.tensor.matmul(
                        pv[:],
                        pts[qc][kb // 2][:, col0:col0 + 128],
                        vo[kb // 4][:, kb % 4, :],
                        start=(kb == 0),
                        stop=(kb == NKB - 1),
                    )
                if kb == NKB - 1:
                    for qt_i in range(NQT):
                        pv = chains[(qc, qt_i)]
                        rcp = outp.tile([128, 1], f32, tag="rcp", name="rcp")
                        nc.vector.reciprocal(rcp[:], pv[:, D:D + 1])
                        ot = outp.tile([128, D], f32, tag="ot", name="ot")
                        nc.vector.tensor_scalar_mul(ot[:], pv[:, 0:D], rcp[:])
                        row0 = qc * 512 + qt_i * 128
                        nc.sync.dma_start(
                            out=out_ext[row0:row0 + 128, :], in_=ot[:]
                        )

            if variant == "ilv":
                LAG = 2
                BURST = 8  # drain PV in bursts, limits LDW mode flips
                queue = []
                started = 0
                for qc in range(NQC):
                    for kbp in range(NKB // 2):
                        st_step(qc, kbp)
                        started += 1
                        queue.append((qc, 2 * kbp))
                        queue.append((qc, 2 * kbp + 1))
                        if started > LAG and len(queue) >= 2 * LAG + BURST:
                            for _ in range(BURST):
                                pv_kb(*queue.pop(0))
                while queue:
                    pv_kb(*queue.pop(0))
            elif variant in ("stag3", "stag4", "stag5") or variant.startswith("stag3L"):
                # Two lagged chunk-0 chains during chunk-0 STs; chunk-1's
                # first chain lagged during chunk-1 STs alongside chunk-0's
                # remaining chains. At most 2 PSUM chains live at once.
                pos = {}

                def adv(qc, qt_i, n, cap):
                    p0 = pos.get((qc, qt_i), 0)
                    p1 = min(p0 + n, cap, NKB)
                    if p1 <= p0:
                        return
                    if p0 == 0:
                        chains[(qc, qt_i)] = pvp.tile(
                            [128, D + 1], f32, tag="pv", name="pv"
                        )
                    pv = chains[(qc, qt_i)]
                    for kb in range(p0, p1):
                        col0 = (kb % 2) * 512 + qt_i * 128
                        nc.tensor.matmul(
                            pv[:],
                            pts[qc][kb // 2][:, col0:col0 + 128],
                            vo[kb // 4][:, kb % 4, :],
                            start=(kb == 0),
                            stop=(kb == NKB - 1),
                        )
                    pos[(qc, qt_i)] = p1
                    if p1 == NKB:
                        rcp = outp.tile([128, 1], f32, tag="rcp", name="rcp")
                        nc.vector.reciprocal(rcp[:], pv[:, D:D + 1])
                        ot = outp.tile([128, D], f32, tag="ot", name="ot")
                        nc.vector.tensor_scalar_mul(ot[:], pv[:, 0:D], rcp[:])
                        row0 = qc * 512 + qt_i * 128
                        nc.sync.dma_start(
                            out=out_ext[row0:row0 + 128, :], in_=ot[:]
                        )

                lag_a, lag_b = 4, 6
                if variant.startswith("stag3L"):
                    lag_a, lag_b = [int(x) for x in variant[6:].split("_")]
                for kbp in range(NKB // 2):
                    st_step(0, kbp)
                    ready = 2 * (kbp + 1)
                    adv(0, 0, 2, ready - lag_a)
                    adv(0, 1, 2, ready - lag_b)
                adv(0, 0, NKB, NKB)
                adv(0, 1, NKB, NKB)
                for step in range(NKB // 2):
                    st_step(1, step)
                    ready = 2 * (step + 1)
                    adv(1, 0, 2, ready - lag_a)
                    n23 = 4 if variant == "stag4" else 2
                    adv(0, 2, n23, NKB)
                    adv(0, 3,
                        n23 if pos.get((0, 2), 0) >= NKB else 0, NKB)
                adv(0, 2, NKB, NKB)
                adv(0, 3, NKB, NKB)
                adv(1, 0, NKB, NKB)
                for qt_i in range(1, NQT):
                    adv(1, qt_i, NKB, NKB)
            elif variant == "stag2":
                # Like stag, but chunk-0's first PV chain is also interleaved
                # (lagged) into the chunk-0 ST stream so the PE is never
                # starved while ACT works through the exps.
                def pv_part2(qc, qt_i, kb0, kb1):
                    if kb0 == 0:
                        chains[(qc, qt_i)] = pvp.tile(
                            [128, D + 1], f32, tag="pv", name="pv"
                        )
                    pv = chains[(qc, qt_i)]
                    for kb in range(kb0, kb1):
                        col0 = (kb % 2) * 512 + qt_i * 128
                        nc.tensor.matmul(
                            pv[:],
                            pts[qc][kb // 2][:, col0:col0 + 128],
                            vo[kb // 4][:, kb % 4, :],
                            start=(kb == 0),
                            stop=(kb == NKB - 1),
                        )
                    if kb1 == NKB:
                        rcp = outp.tile([128, 1], f32, tag="rcp", name="rcp")
                        nc.vector.reciprocal(rcp[:], pv[:, D:D + 1])
                        ot = outp.tile([128, D], f32, tag="ot", name="ot")
                        nc.vector.tensor_scalar_mul(ot[:], pv[:, 0:D], rcp[:])
                        row0 = qc * 512 + qt_i * 128
                        nc.sync.dma_start(
                            out=out_ext[row0:row0 + 128, :], in_=ot[:]
                        )

                # chunk 0: ST stream with chain qt0 lagging 2 kbp behind
                for kbp in range(NKB // 2):
                    st_step(0, kbp)
                    if kbp >= 2:
                        pv_part2(0, 0, (kbp - 2) * 2, (kbp - 2) * 2 + 2)
                pv_part2(0, 0, NKB - 4, NKB)
                # chunk 1 STs interleaved with chains qt1-3 of chunk 0
                # (flat list of 192 MMs, 6 per step, chains stay in order)
                blocks = []
                for qt_i in range(1, NQT):
                    for kb in range(NKB):
                        blocks.append((qt_i, kb))
                bi = 0
                for step in range(NKB // 2):
                    st_step(1, step)
                    take = 6 if step < 31 else len(blocks) - bi
                    for _ in range(take):
                        qt_i, kb = blocks[bi]
                        bi += 1
                        pv_part2(0, qt_i, kb, kb + 1)
                # chunk 1 PV chains
                for qt_i in range(NQT):
                    pv_part2(1, qt_i, 0, NKB)
            elif variant == "stag":
                # qc0: plain ST phase. Then steps interleaving one ST(qc1)
                # pair with 8 contiguous PV(qc0) chain matmuls (chains stay
                # qt-major / single-bank). Finally PV(qc1) qt-major.
                def pv_chain_part(qc, qt_i, kb0, kb1):
                    if kb0 == 0:
                        chains[(qc, qt_i)] = pvp.tile(
                            [128, D + 1], f32, tag="pv", name="pv"
                        )
                    pv = chains[(qc, qt_i)]
                    for kb in range(kb0, kb1):
                        col0 = (kb % 2) * 512 + qt_i * 128
                        nc.tensor.matmul(
                            pv[:],
                            pts[qc][kb // 2][:, col0:col0 + 128],
                            vo[kb // 4][:, kb % 4, :],
                            start=(kb == 0),
                            stop=(kb == NKB - 1),
                        )
                    if kb1 == NKB:
                        rcp = outp.tile([128, 1], f32, tag="rcp", name="rcp")
                        nc.vector.reciprocal(rcp[:], pv[:, D:D + 1])
                        ot = outp.tile([128, D], f32, tag="ot", name="ot")
                        nc.vector.tensor_scalar_mul(ot[:], pv[:, 0:D], rcp[:])
                        row0 = qc * 512 + qt_i * 128
                        nc.sync.dma_start(
                            out=out_ext[row0:row0 + 128, :], in_=ot[:]
                        )

                for kbp in range(NKB // 2):
                    st_step(0, kbp)
                for step in range(32):
                    st_step(1, step)
                    qt_i, seg = divmod(step, 8)
                    pv_chain_part(0, qt_i, seg * 8, seg * 8 + 8)
                for qt_i in range(NQT):
                    pv_chain_part(1, qt_i, 0, NKB)
            elif variant == "seq4":
                # per-chunk: all ST, then kb-major PV (all 4 chains advance
                # together so pt slots release early for the next chunk)
                for qc in range(NQC):
                    for kbp in range(NKB // 2):
                        st_step(qc, kbp)
                    for kb in range(NKB):
                        pv_kb(qc, kb)
            else:
                # sequential: all ST of a chunk, then qt-major PV chains
                for qc in range(NQC):
                    for kbp in range(NKB // 2):
                        st_step(qc, kbp)
                    for qt_i in range(NQT):
                        pv = pvp.tile([128, D + 1], f32, tag="pv", name="pv")
                        for kb in range(NKB):
                            col0 = (kb % 2) * 512 + qt_i * 128
                            nc.tensor.matmul(
                                pv[:],
                                pts[qc][kb // 2][:, col0:col0 + 128],
                                vo[kb // 4][:, kb % 4, :],
                                start=(kb == 0),
                                stop=(kb == NKB - 1),
                            )
                        rcp = outp.tile([128, 1], f32, tag="rcp", name="rcp")
                        nc.vector.reciprocal(rcp[:], pv[:, D:D + 1])
                        ot = outp.tile([128, D], f32, tag="ot", name="ot")
                        nc.vector.tensor_scalar_mul(ot[:], pv[:, 0:D], rcp[:])
                        row0 = qc * 512 + qt_i * 128
                        nc.sync.dma_start(
                            out=out_ext[row0:row0 + 128, :], in_=ot[:]
                        )


def _get_nc(repeat=1, variant=None):
    if variant is None:
        variant = DEFAULT_VARIANT
    key = f"nc{repeat}-{variant}"
    if key not in _CACHE:
        if variant.startswith("vb"):
            # vb[_L<lag>][_<order><batch>][_w<nwarm>]
            lag, order, batch, nwarm = 3, "jh", 2, 12
            for part in variant.split("_")[1:]:
                if part.startswith("L"):
                    lag = int(part[1:])
                elif part.startswith("w"):
                    nwarm = int(part[1:])
                elif part[:2] in ("jh", "hj"):
                    order, batch = part[:2], int(part[2:])
            _CACHE[key] = _build_vb(repeat, lag, order, batch, nwarm)
        else:
            _CACHE[key] = _build(repeat, variant)
    return _CACHE[key]


def run(inputs, trace=False):
    """Run on 8 cores; returns (full_output, BassKernelResults)."""
    from concourse.bass_utils import run_bass_kernel_spmd

    Q = np.ascontiguousarray(np.asarray(inputs["Q"], dtype=np.float32))
    K = np.ascontiguousarray(np.asarray(inputs["K"], dtype=np.float32))
    V = np.ascontiguousarray(np.asarray(inputs["V"], dtype=np.float32))

    nc = _get_nc()
    in_maps = [
        {"Q": Q[i * SHARD:(i + 1) * SHARD], "K": K, "V": V}
        for i in range(N_CORES)
    ]
    res = run_bass_kernel_spmd(
        nc, in_maps, core_ids=list(range(N_CORES)), trace=trace
    )
    out = np.concatenate([res.results[i]["out"] for i in range(N_CORES)], axis=0)
    return out, res


def kernel(**inputs) -> np.ndarray:
    import time

    last_err = None
    for attempt in range(3):
        try:
            out, _ = run(inputs, trace=False)
            return out
        except Exception as e:  # transient axon/device wedge - retry
            last_err = e
            time.sleep(15 * (attempt + 1))
    raise last_err



# revision 33
# speedup vs baseline: 1.4307x; 1.4307x over previous
"""Distributed attention kernel for Trainium2 (8 NeuronCores).

Problem: softmax(Q @ K.T / sqrt(S)) @ V with S=8192, D=256, fp32 I/O.
Note the reference scales by sqrt(K.shape[-2]) = sqrt(S), NOT sqrt(D).

Sharding: Q rows split across 8 cores (1024 rows each); K, V replicated.
No collectives needed - each core computes its output rows independently.

Per-core algorithm:
  - DMA-load Q, K, V with inline f32->bf16 cast (SWDGE).
  - PE-transpose Q and K into Q^T [D, 1024] and K^T [D, S] (the score
    matmul contracts over D, so both operands need D on partitions); the
    PSUM->SBUF copies cast them to fp8e4m3, laid out [128, 2, n] for
    DoubleRow's (partition, k-tile) contraction.
  - Scores are computed TRANSPOSED in one fp8 DoubleRow matmul per key
    block (contraction 256 at 2 MACs/cell/cycle): S^T[keys, q] = K @ Q^T.
    The exp output P^T = exp(S^T * scale) (bf16, in SBUF) is directly the
    stationary operand (lhsT) of the P @ V matmul - no P transpose needed.
  - Scores are ~N(0, 1/32) so softmax needs no max subtraction; fp8
    score error (~0.7% on softmax weights) dominates the output rel_err
    of ~7e-3, well under the 2e-2 gate.
  - V (bf16) gets a ones-column appended: P @ [V | 1] accumulates the
    unnormalized output and the softmax denominator in one PSUM chain.
  - Normalize with a per-partition reciprocal multiply, DMA out fp32.
  - Schedule ("stag3"/"stag4"): chunk-0's score stream runs with its
    first TWO PV chains lagging 2-3 key-block-pairs behind the exps;
    chunk-1's score stream then interleaves with chunk-0's remaining
    chains and chunk-1's own first chain. At most 2 PV chains are live
    at once (single-PSUM-bank accumulation chains are HW-measured much
    faster than multi-chain bank-cycling patterns). Filling the
    exp-paced windows with PV matmuls this way cut ~25%+ on hardware
    versus the phase-sequential schedule, invisible to the cost model.
  - Final default "pstag4w12" adds: (a) 12 warm-up matmuls during the
    DMA lead-in (engages the PE p-state ramp; 40 was measured slower -
    warmups run at the low clock and delay the first real work),
    (b) prologue reorder - K group 0 is loaded and PE-transposed before
    Q's transposes and Q is transposed g-major, so the first score
    matmul starts ~2-3us earlier, (c) the stag4 chain schedule (chunk-0
    qt2/qt3 chains drain 4 key-blocks per step during chunk-1's score
    stream).
  - Rejected on HW measurement: XBAR-DMA transposes of Q/K (the
    per-dispatch HWDGE/ucode cost outweighs the freed PE cycles) and a
    V-stationary PV restructure with a DVE-tree softmax denominator
    (fewer weight loads but slower chain/bank pattern on HW).
"""

import numpy as np

S = 8192
D = 256
N_CORES = 8
SHARD = S // N_CORES  # 1024 query rows per core

import os as _os

# pstag4w12: stag4 chain schedule + K-group-0-first/g-major-Q prologue +
# 12 warm-up matmuls. Selected on hardware via interleaved median-of-m16
# A/B against stag3/stag3w12/pstag3w12/stag4w12.
DEFAULT_VARIANT = _os.environ.get("KVAR", "pstag4w12")

_CACHE = {}


def _build_vb(repeat=1, lag=3, order="hj", batch=2, nwarm=12):
    """V-stationary rewrite ("vb"):

    - Q/K transposed by the DMA XBAR (dma_start_transpose), not the PE.
    - P@V uses V blocks as the stationary operand and pt (=exp scores,
      [keys, queries]) as the moving operand: 4 psum chains (2 d-chunks x
      2 q-halves) accumulate out^T over all 64 key blocks. 256 weight
      loads instead of 512, and no ones-column.
    - softmax denominator: DVE sums pt tiles via a bf16 pair/quad/oct
      tree (4x DVE mode) + one f32 accumulate per 8 kb into
      acc[128, 1024]; at the end 8 tiny matmuls (acc tile stationary,
      ones column moving) give denom transposed [128q, 8], reciprocal on
      DVE, applied post-transpose as a per-partition ACT scale.
    - output: psum out^T -> bf16 sbuf -> XBAR transpose -> scale ->
      casting DMA store (bf16 -> f32).
    """
    import concourse.mybir as mybir
    import concourse.tile as tile
    from concourse import bacc

    f32 = mybir.dt.float32
    bf16 = mybir.dt.bfloat16
    f8 = mybir.dt.float8e4
    AF = mybir.ActivationFunctionType

    SCALE = 1.0 / float(np.sqrt(np.float32(S)))
    NKB = S // 128  # 64 key blocks
    KG = 8          # max key blocks per staged K group
    NKG = None      # group list built in _emit_vb
    KPF = 3         # K groups prefetched ahead (beyond the current)
    VPF = 2         # V slabs (8 kb each) prefetched ahead

    nc = bacc.Bacc()
    q_ext = nc.dram_tensor("Q", [SHARD, D], f32, kind="ExternalInput")
    k_ext = nc.dram_tensor("K", [S, D], f32, kind="ExternalInput")
    v_ext = nc.dram_tensor("V", [S, D], f32, kind="ExternalInput")
    out_ext = nc.dram_tensor("out", [SHARD, D], f32, kind="ExternalOutput")

    with tile.TileContext(nc) as tc:
        with (
            tc.tile_pool(name="singles", bufs=1) as singles,
            tc.tile_pool(name="kstage", bufs=4) as kstage,
            tc.tile_pool(name="kbst", bufs=4) as kbst,
            tc.tile_pool(name="ptp", bufs=lag + 5) as ptp,
            tc.tile_pool(name="trees", bufs=1) as trees,
            tc.tile_pool(name="outp", bufs=1) as outp,
            tc.tile_pool(name="stp", bufs=2, space="PSUM") as stp,
            tc.tile_pool(name="pvp", bufs=1, space="PSUM") as pvp,
        ):
            warm = singles.tile([128, 128], bf16, tag="warm", name="warm")
            nc.vector.memset(warm[:], 0.0)
            ones_col = singles.tile([128, 1], f32, tag="ones", name="ones")
            nc.vector.memset(ones_col[:], 1.0)

            for _rep in range(repeat):
                _emit_vb(nc, tc, singles, kstage, kbst, ptp, trees, outp,
                         stp, pvp, warm, ones_col, q_ext, k_ext, v_ext,
                         out_ext, mybir, SCALE, NKB, KG, NKG, KPF, VPF,
                         lag, order, batch, nwarm)

    nc.finalize()
    return nc


def _emit_vb(nc, tc, singles, kstage, kbst, ptp, trees, outp, stp, pvp,
             warm, ones_col, q_ext, k_ext, v_ext, out_ext,
             mybir, SCALE, NKB, KG, NKG, KPF, VPF, lag, order, batch, nwarm):
    f32 = mybir.dt.float32
    bf16 = mybir.dt.bfloat16
    f8 = mybir.dt.float8e4
    AF = mybir.ActivationFunctionType

    acc = singles.tile([128, SHARD], f32, tag="acc", name="acc")
    nc.vector.memset(acc[:], 0.0)

    # ---- warm-up matmuls: engage the PE clock while input DMAs land ----
    if nwarm:
        wp = stp.tile([128, 1024], f32, tag="st", name="wp")
        for _ in range(nwarm):
            nc.tensor.matmul(wp[:, 0:128], warm[:], warm[:],
                             start=True, stop=True)

    # ---- Q: casting load + one batched XBAR transpose + fp8 cast ----
    # dma_start_transpose with out [128, X, 128] (3D) transposes each
    # 128-col slab x of the 2D input independently:
    #   out[a, x, y] = in[y, x*128 + a]
    # With in = qs [128q, (n d) = n*256 + c*128 + a] the slab index is
    # x = n*2 + c, so out = qtb [128, n, c, 128] (c INNER) gives
    # qtb[a, n, c, y] = Q[n*128 + y, c*128 + a] — the transposed layout.
    kt8 = singles.tile([128, 2, S], f8, tag="kt8", name="kt8")
    k_all = k_ext.rearrange("(b p) d -> p b d", p=128)
    vt = singles.tile([128, NKB, D], bf16, tag="vt", name="vt")
    v_re = v_ext.rearrange("(b p) d -> p b d", p=128)

    def emit_k_group(b0, nb):
        ks = kstage.tile([128, KG, D], bf16, tag="ks", name="ks")
        nc.gpsimd.dma_start(out=ks[:, 0:nb, :], in_=k_all[:, b0:b0 + nb, :])
        ktb = kbst.tile([128, KG, 2, 128], bf16, tag="ktb", name="ktb")
        nc.sync.dma_start_transpose(ktb[:, 0:nb, :, :], ks[:, 0:nb, :])
        for c in range(2):
            nc.vector.tensor_copy(
                kt8[:, c, b0 * 128:(b0 + nb) * 128], ktb[:, 0:nb, c, :]
            )

    def emit_v_slab(s):
        nc.gpsimd.dma_start(
            out=vt[:, s * 16:(s + 1) * 16, :],
            in_=v_re[:, s * 16:(s + 1) * 16, :],
        )

    # group list: first two groups small so ST(0) starts ASAP
    kgroups = [(0, 4), (4, 4)] + [(8 * t, 8) for t in range(1, 8)]
    kg_start = {b0: gi for gi, (b0, _nb) in enumerate(kgroups)}
    kg_next = 0

    def emit_k_groups_until(gi):
        nonlocal kg_next
        while kg_next <= gi and kg_next < len(kgroups):
            emit_k_group(*kgroups[kg_next])
            kg_next += 1

    # K group 0 first: its dma->xbar->cast chain gates ST(0).
    emit_k_groups_until(0)

    # ---- Q: casting load + one batched XBAR transpose + fp8 cast ----
    qs = singles.tile([128, SHARD // 128, D], bf16, tag="qs", name="qs")
    q_re = q_ext.rearrange("(n p) d -> p n d", p=128)
    nc.gpsimd.dma_start(out=qs[:], in_=q_re[:])
    qtb = singles.tile([128, SHARD // 128, 2, 128], bf16, tag="qtb",
                       name="qtb")
    nc.sync.dma_start_transpose(qtb[:], qs[:])
    qt8 = singles.tile([128, 2, SHARD], f8, tag="qt8", name="qt8")
    for c in range(2):
        nc.vector.tensor_copy(qt8[:, c, :], qtb[:, :, c, :])

    emit_k_groups_until(KPF - 1)
    for s in range(VPF):
        emit_v_slab(s)

    # ---- main stream ----
    pts = {}
    pairs, quads = [], []
    ot = [[None, None], [None, None]]  # [c][qh] psum chains

    def st_step(kb):
        st = stp.tile([128, 1024], f32, tag="st", name="st")
        for qh in range(2):
            nc.tensor.matmul(
                st[:, qh * 512:(qh + 1) * 512],
                kt8[:, :, kb * 128:(kb + 1) * 128],
                qt8[:, :, qh * 512:(qh + 1) * 512],
                start=True,
                stop=True,
                perf_mode=mybir.MatmulPerfMode.DoubleRow,
            )
        pt = ptp.tile([128, 1024], bf16, tag="pt", name="pt")
        nc.scalar.activation(pt[:], st[:], AF.Exp, scale=SCALE)
        pts[kb] = pt

    def pv_one(c, qh, j):
        if j == 0:
            ot[c][qh] = pvp.tile([128, 512], f32, tag=f"ot{c}{qh}",
                                 name=f"ot{c}{qh}")
        nc.tensor.matmul(
            ot[c][qh][:],
            vt[:, j, c * 128:(c + 1) * 128],
            pts[j][:, qh * 512:(qh + 1) * 512],
            start=(j == 0),
            stop=(j == NKB - 1),
        )

    def pv_emit(js):
        if order == "jh":
            for c in range(2):
                for j in js:
                    for qh in range(2):
                        pv_one(c, qh, j)
        else:  # "hj"
            for c in range(2):
                for qh in range(2):
                    for j in js:
                        pv_one(c, qh, j)

    pv_next = 0
    for kb in range(NKB):
        if kb in kg_start:
            emit_k_groups_until(kg_start[kb] + KPF)
        if kb % 16 == 0:
            s = kb // 16 + VPF
            if s < NKB // 16:
                emit_v_slab(s)
        st_step(kb)
        # bf16 sum tree (DVE 4x mode) -> one f32 accumulate per 8 kb;
        # bf16 rounding on <=8-term partials adds ~6e-4 denominator error.
        if kb % 2 == 1:
            pr = trees.tile([128, 1024], bf16, tag=f"pr{(kb // 2) % 2}",
                            name="pr")
            nc.vector.tensor_tensor(pr[:], pts[kb - 1][:], pts[kb][:],
                                    mybir.AluOpType.add)
            pairs.append(pr)
        if kb % 4 == 3:
            qd = trees.tile([128, 1024], bf16, tag=f"qd{(kb // 4) % 2}",
                            name="qd")
            nc.vector.tensor_tensor(qd[:], pairs[-2][:], pairs[-1][:],
                                    mybir.AluOpType.add)
            quads.append(qd)
        if kb % 8 == 7:
            oc = trees.tile([128, 1024], bf16, tag="oc", name="oc")
            nc.vector.tensor_tensor(oc[:], quads[-2][:], quads[-1][:],
                                    mybir.AluOpType.add)
            nc.vector.tensor_tensor(acc[:], acc[:], oc[:],
                                    mybir.AluOpType.add)
        ready = kb - lag
        while pv_next <= ready - (batch - 1):
            pv_emit(list(range(pv_next, pv_next + batch)))
            pv_next += batch
    while pv_next < NKB:
        n = min(batch, NKB - pv_next)
        pv_emit(list(range(pv_next, pv_next + n)))
        pv_next += n

    # ---- tail: denominator, reciprocal, cast, transpose, scale, store ----
    dt = stp.tile([128, 1024], f32, tag="st", name="dt")
    for t in range(8):
        nc.tensor.matmul(
            dt[:, t:t + 1],
            acc[:, t * 128:(t + 1) * 128],
            ones_col[:],
            start=True,
            stop=True,
        )
    rcpT = outp.tile([128, 8], f32, tag="rcp", name="rcp")
    nc.vector.reciprocal(rcpT[:], dt[:, 0:8])

    ot_bf = outp.tile([128, 2, 1024], bf16, tag="otb", name="otb")
    for qh in range(2):
        nc.vector.tensor_copy(ot_bf[:, 0, qh * 512:(qh + 1) * 512],
                              ot[0][qh][:])
        nc.scalar.copy(ot_bf[:, 1, qh * 512:(qh + 1) * 512],
                       ot[1][qh][:])
    o = outp.tile([128, 8, D], bf16, tag="o", name="o")
    for c in range(2):
        nc.sync.dma_start_transpose(
            o[:, :, c * 128:(c + 1) * 128], ot_bf[:, c, :]
        )
    o2 = outp.tile([128, 8, D], bf16, tag="o2", name="o2")
    for t in range(8):
        nc.scalar.mul(o2[:, t, :], o[:, t, :], rcpT[:, t:t + 1])
    out_re = out_ext.rearrange("(t p) d -> p t d", p=128)
    nc.gpsimd.dma_start(out=out_re[:], in_=o2[:])


def _build(repeat=1, variant="stag3"):
    import concourse.mybir as mybir
    import concourse.tile as tile
    from concourse import bacc
    from concourse.masks import make_identity

    f32 = mybir.dt.float32
    bf16 = mybir.dt.bfloat16
    f8 = mybir.dt.float8e4
    AF = mybir.ActivationFunctionType

    SCALE = 1.0 / float(np.sqrt(np.float32(S)))

    NKB = S // 128      # 64 key blocks
    NQC = SHARD // 512  # 2 query chunks per core
    NQT = 4             # 128-row query tiles per chunk

    nc = bacc.Bacc()
    q_ext = nc.dram_tensor("Q", [SHARD, D], f32, kind="ExternalInput")
    k_ext = nc.dram_tensor("K", [S, D], f32, kind="ExternalInput")
    v_ext = nc.dram_tensor("V", [S, D], f32, kind="ExternalInput")
    out_ext = nc.dram_tensor("out", [SHARD, D], f32, kind="ExternalOutput")

    is_sx = variant.startswith("sx")
    with tile.TileContext(nc) as tc:
        with (
            tc.tile_pool(name="singles", bufs=1) as singles,
            tc.tile_pool(
                name="kstage", bufs=5 if variant.endswith("k") else 3
            ) as kstage,
            tc.tile_pool(name="kbst", bufs=2) as kbst,
            tc.tile_pool(name="ptp", bufs=1) as ptp,
            tc.tile_pool(name="outp", bufs=4) as outp,
            tc.tile_pool(
                name="stp",
                bufs=3 if (variant == "stag5" or is_sx) else 2,
                space="PSUM",
            ) as stp,
            tc.tile_pool(
                name="pvp",
                bufs=4 if variant == "ilv" else 2,
                space="PSUM",
            ) as pvp,
            tc.tile_pool(name="trp", bufs=2, space="PSUM") as trp,
        ):
            ident = singles.tile([128, 128], bf16, tag="ident", name="ident")
            make_identity(nc, ident)
            if variant.endswith("k"):
                variant = variant[:-1]
            import re as _re
            wm = _re.search(r"w(\d*)$", variant)
            if wm:
                # Warm-up matmuls: fill the lead-in while the first input
                # DMAs land, and engage the HAM clock-gate (transposes
                # alone don't) so early real matmuls run at full rate.
                # Too many delays the first real work (they run at the low
                # p-state): keep the count small.
                nwarm = int(wm.group(1) or "40")
                if is_sx:
                    wp = stp.tile([128, 1024], f32, tag="st", name="wp")
                    for _ in range(nwarm):
                        nc.tensor.matmul(wp[:, 0:128], ident[:], ident[:],
                                         start=True, stop=True)
                else:
                    wp = trp.tile([128, 128], f32, tag="tr", name="wp")
                    for _ in range(nwarm):
                        nc.tensor.matmul(wp[:], ident[:], ident[:],
                                         start=True, stop=True)
                variant = variant[:wm.start()]
            for _rep in range(repeat):
                _emit_body(nc, tc, singles, kstage, ptp, outp, stp, pvp,
                           trp, ident, q_ext, k_ext, v_ext, out_ext,
                           mybir, SCALE, NKB, NQC, NQT, variant,
                           kbst=kbst)

    nc.finalize()
    return nc


def _emit_body(nc, tc, singles, kstage, ptp, outp, stp, pvp,
               trp, ident, q_ext, k_ext, v_ext, out_ext,
               mybir, SCALE, NKB, NQC, NQT, variant="stag3", kbst=None):
    f32 = mybir.dt.float32
    bf16 = mybir.dt.bfloat16
    f8 = mybir.dt.float8e4
    AF = mybir.ActivationFunctionType

    is_sx = variant.startswith("sx")
    if is_sx:
        # sx<sched>: xbar transposes + a stag-family schedule (default stag3)
        variant = variant[2:] or "stag3"
    pro = variant.startswith("p")
    if pro:
        # prologue-optimized: K group 0 loads+transposes before Q's
        # transposes, Q transposed g-major, so ST(0) starts sooner.
        variant = variant[1:]

    if True:
        if True:
            # ---- Q: load (cast) + transpose into QT [128, 2, SHARD] ----
            # QT[p, c, q] = Q[q, c*128+p]; dim1 = d-chunk for DoubleRow's
            # (p, t) contraction layout.
            qs = singles.tile([128, SHARD // 128, D], bf16, tag="qs", name="qs")
            q_re = q_ext.rearrange("(n p) d -> p n d", p=128)
            st_dt = bf16 if variant == "seqbf" else f8
            qt8 = singles.tile([128, 2, SHARD], st_dt, tag="qt8", name="qt8")

            kt8_early = None
            if pro and not is_sx:
                kt8_early = singles.tile([128, 2, S], st_dt, tag="kt8",
                                         name="kt8")
                k_all_e = k_ext.rearrange("(b p) d -> p b d", p=128)
                ks0 = kstage.tile([128, 8, D], bf16, tag="ks", name="ks")
                nc.gpsimd.dma_start(out=ks0[:, 0:4, :], in_=k_all_e[:, 0:4, :])
                nc.gpsimd.dma_start(out=qs[:, 0:4, :], in_=q_re[:, 0:4, :])
                nc.gpsimd.dma_start(out=qs[:, 4:8, :], in_=q_re[:, 4:8, :])
                for c in range(2):
                    tr = trp.tile([128, 512], bf16, tag="tr", name="tr")
                    for j in range(4):
                        nc.tensor.transpose(
                            tr[:, j * 128:(j + 1) * 128],
                            ks0[:, j, c * 128:(c + 1) * 128],
                            ident,
                        )
                    nc.vector.tensor_copy(kt8_early[:, c, 0:512], tr[:])
                # Q transposes g-major: qt8 columns for the first query
                # chunk (both c) land first
                for g in range(SHARD // 512):
                    for c in range(2):
                        tr = trp.tile([128, 512], bf16, tag="tr", name="tr")
                        for j in range(4):
                            qi = g * 4 + j
                            nc.tensor.transpose(
                                tr[:, j * 128:(j + 1) * 128],
                                qs[:, qi, c * 128:(c + 1) * 128],
                                ident,
                            )
                        nc.vector.tensor_copy(
                            qt8[:, c, g * 512:(g + 1) * 512], tr[:]
                        )
            if is_sx:
                nc.gpsimd.dma_start(out=qs[:], in_=q_re[:])
                qtb = kbst.tile([128, SHARD // 128, 2, 128], bf16,
                                tag="ktb", name="qtb")
                nc.sync.dma_start_transpose(qtb[:], qs[:])
                for c in range(2):
                    nc.vector.tensor_copy(qt8[:, c, :], qtb[:, :, c, :])
            elif not pro:
                nc.gpsimd.dma_start(out=qs[:, 0:4, :], in_=q_re[:, 0:4, :])
                nc.gpsimd.dma_start(out=qs[:, 4:8, :], in_=q_re[:, 4:8, :])
                for c in range(2):
                    for g in range(SHARD // 512):
                        tr = (trp.tile([128, 512], bf16, tag="tr", name="tr")
                              if variant in ("seq", "seqp", "stag", "stag2", "stag3", "stag4") else
                              stp.tile([128, 512], bf16, tag="st", name="tr"))
                        for j in range(4):
                            qi = g * 4 + j
                            nc.tensor.transpose(
                                tr[:, j * 128:(j + 1) * 128],
                                qs[:, qi, c * 128:(c + 1) * 128],
                                ident,
                            )
                        nc.vector.tensor_copy(
                            qt8[:, c, g * 512:(g + 1) * 512], tr[:]
                        )

            # ---- K: load (cast) + transpose into KT [128, 2, S] ----
            kt8 = (kt8_early if kt8_early is not None else
                   singles.tile([128, 2, S], st_dt, tag="kt8", name="kt8"))
            k_all = k_ext.rearrange("(b p) d -> p b d", p=128)
            groups = [(0, 4), (4, 4)] + [(8 * t, 8) for t in range(1, 8)]
            if pro and not is_sx:
                groups = groups[1:]
            for b_start, nb in groups:
                ks = kstage.tile([128, 8, D], bf16, tag="ks", name="ks")
                nc.gpsimd.dma_start(
                    out=ks[:, 0:nb, :],
                    in_=k_all[:, b_start:b_start + nb, :],
                )
                if is_sx:
                    ktb = kbst.tile([128, 8, 2, 128], bf16, tag="ktb",
                                    name="ktb")
                    nc.sync.dma_start_transpose(
                        ktb[:, 0:nb, :, :], ks[:, 0:nb, :]
                    )
                    for c in range(2):
                        nc.vector.tensor_copy(
                            kt8[:, c, b_start * 128:(b_start + nb) * 128],
                            ktb[:, 0:nb, c, :],
                        )
                    continue
                for c in range(2):
                    for g in range(nb // 4):
                        tr = (trp.tile([128, 512], bf16, tag="tr", name="tr")
                          if variant in ("seq", "seqp", "stag", "stag2", "stag3", "stag4") else
                          stp.tile([128, 512], bf16, tag="st", name="tr"))
                        for j in range(4):
                            n = g * 4 + j
                            nc.tensor.transpose(
                                tr[:, j * 128:(j + 1) * 128],
                                ks[:, n, c * 128:(c + 1) * 128],
                                ident,
                            )
                        b0 = b_start + g * 4
                        nc.vector.tensor_copy(
                            kt8[:, c, b0 * 128:(b0 + 4) * 128], tr[:]
                        )

            # ---- V: load (cast) with a ones-column appended ----
            vo = []
            v_re = v_ext.rearrange("(t n p) d -> t p n d", p=128, n=4)
            for t in range(S // 512):
                vt = singles.tile([128, 4, D + 1], bf16, tag=f"vo{t}", name=f"vo{t}")
                nc.vector.memset(vt[:, :, D:D + 1], 1.0)
                nc.gpsimd.dma_start(out=vt[:, :, 0:D], in_=v_re[t])
                vo.append(vt)

            # ---- main: ST stream interleaved with kb-major PV chains ----
            # PV for key block kb is emitted LAG key-block-pairs after its
            # exp, so the PE always has PV matmuls to run between ST
            # matmuls instead of stalling at the exp production rate.
            pts = {0: [], 1: []}
            chains = {}

            def st_step(qc, kbp):
                st = stp.tile([128, 1024], f32, tag="st", name="st")
                ptag = (f"pt{qc}_{kbp}" if variant.startswith(("seqp", "stag"))
                        else f"pt{kbp}")
                pt = ptp.tile([128, 1024], bf16, tag=ptag, name=ptag)
                for half in range(2):
                    kb = kbp * 2 + half
                    if variant == "seqbf":
                        for c in range(2):
                            nc.tensor.matmul(
                                st[:, half * 512:(half + 1) * 512],
                                kt8[:, c, kb * 128:(kb + 1) * 128],
                                qt8[:, c, qc * 512:(qc + 1) * 512],
                                start=(c == 0),
                                stop=(c == 1),
                            )
                    else:
                        nc.tensor.matmul(
                            st[:, half * 512:(half + 1) * 512],
                            kt8[:, :, kb * 128:(kb + 1) * 128],
                            qt8[:, :, qc * 512:(qc + 1) * 512],
                            start=True,
                            stop=True,
                            perf_mode=mybir.MatmulPerfMode.DoubleRow,
                        )
                nc.scalar.activation(pt[:], st[:], AF.Exp, scale=SCALE)
                pts[qc].append(pt)

            def pv_kb(qc, kb):
                for qt_i in range(NQT):
                    if kb == 0:
                        chains[(qc, qt_i)] = pvp.tile(
                            [128, D + 1], f32, tag="pv", name="pv"
                        )
                    pv = chains[(qc, qt_i)]
                    col0 = (kb % 2) * 512 + qt_i * 128
                    nc.tensor.matmul(
                        pv[:],
                        pts[qc][kb // 2][:, col0:col0 + 128],
                        vo[kb // 4][:, kb % 4, :],
                        start=(kb == 0),
                        stop=(kb == NKB - 1),
                    )
                if kb == NKB - 1:
                    for qt_i in range(NQT):
                        pv = chains[(qc, qt_i)]
                        rcp = outp.tile([128, 1], f32, tag="rcp", name="rcp")
                        nc.vector.reciprocal(rcp[:], pv[:, D:D + 1])
                        ot = outp.tile([128, D], f32, tag="ot", name="ot")
                        nc.vector.tensor_scalar_mul(ot[:], pv[:, 0:D], rcp[:])
                        row0 = qc * 512 + qt_i * 128
                        nc.sync.dma_start(
                            out=out_ext[row0:row0 + 128, :], in_=ot[:]
                        )

            if variant == "ilv":
                LAG = 2
                BURST = 8  # drain PV in bursts, limits LDW mode flips
                queue = []
                started = 0
                for qc in range(NQC):
                    for kbp in range(NKB // 2):
                        st_step(qc, kbp)
                        started += 1
                        queue.append((qc, 2 * kbp))
                        queue.append((qc, 2 * kbp + 1))
                        if started > LAG and len(queue) >= 2 * LAG + BURST:
                            for _ in range(BURST):
                                pv_kb(*queue.pop(0))
                while queue:
                    pv_kb(*queue.pop(0))
            elif variant in ("stag3", "stag4", "stag5") or variant.startswith("stag3L"):
                # Two lagged chunk-0 chains during chunk-0 STs; chunk-1's
                # first chain lagged during chunk-1 STs alongside chunk-0's
                # remaining chains. At most 2 PSUM chains live at once.
                pos = {}

                def adv(qc, qt_i, n, cap):
                    p0 = pos.get((qc, qt_i), 0)
                    p1 = min(p0 + n, cap, NKB)
                    if p1 <= p0:
                        return
                    if p0 == 0:
                        chains[(qc, qt_i)] = pvp.tile(
                            [128, D + 1], f32, tag="pv", name="pv"
                        )
                    pv = chains[(qc, qt_i)]
                    for kb in range(p0, p1):
                        col0 = (kb % 2) * 512 + qt_i * 128
                        nc.tensor.matmul(
                            pv[:],
                            pts[qc][kb // 2][:, col0:col0 + 128],
                            vo[kb // 4][:, kb % 4, :],
                            start=(kb == 0),
                            stop=(kb == NKB - 1),
                        )
                    pos[(qc, qt_i)] = p1
                    if p1 == NKB:
                        rcp = outp.tile([128, 1], f32, tag="rcp", name="rcp")
                        nc.vector.reciprocal(rcp[:], pv[:, D:D + 1])
                        ot = outp.tile([128, D], f32, tag="ot", name="ot")
                        nc.vector.tensor_scalar_mul(ot[:], pv[:, 0:D], rcp[:])
                        row0 = qc * 512 + qt_i * 128
                        nc.sync.dma_start(
                            out=out_ext[row0:row0 + 128, :], in_=ot[:]
                        )

                lag_a, lag_b = 4, 6
                if variant.startswith("stag3L"):
                    lag_a, lag_b = [int(x) for x in variant[6:].split("_")]
                for kbp in range(NKB // 2):
                    st_step(0, kbp)
                    ready = 2 * (kbp + 1)
                    adv(0, 0, 2, ready - lag_a)
                    adv(0, 1, 2, ready - lag_b)
                adv(0, 0, NKB, NKB)
                adv(0, 1, NKB, NKB)
                for step in range(NKB // 2):
                    st_step(1, step)
                    ready = 2 * (step + 1)
                    adv(1, 0, 2, ready - lag_a)
                    n23 = 4 if variant == "stag4" else 2
                    adv(0, 2, n23, NKB)
                    adv(0, 3,
                        n23 if pos.get((0, 2), 0) >= NKB else 0, NKB)
                adv(0, 2, NKB, NKB)
                adv(0, 3, NKB, NKB)
                adv(1, 0, NKB, NKB)
                for qt_i in range(1, NQT):
                    adv(1, qt_i, NKB, NKB)
            elif variant == "stag2":
                # Like stag, but chunk-0's first PV chain is also interleaved
                # (lagged) into the chunk-0 ST stream so the PE is never
                # starved while ACT works through the exps.
                def pv_part2(qc, qt_i, kb0, kb1):
                    if kb0 == 0:
                        chains[(qc, qt_i)] = pvp.tile(
                            [128, D + 1], f32, tag="pv", name="pv"
                        )
                    pv = chains[(qc, qt_i)]
                    for kb in range(kb0, kb1):
                        col0 = (kb % 2) * 512 + qt_i * 128
                        nc.tensor.matmul(
                            pv[:],
                            pts[qc][kb // 2][:, col0:col0 + 128],
                            vo[kb // 4][:, kb % 4, :],
                            start=(kb == 0),
                            stop=(kb == NKB - 1),
                        )
                    if kb1 == NKB:
                        rcp = outp.tile([128, 1], f32, tag="rcp", name="rcp")
                        nc.vector.reciprocal(rcp[:], pv[:, D:D + 1])
                        ot = outp.tile([128, D], f32, tag="ot", name="ot")
                        nc.vector.tensor_scalar_mul(ot[:], pv[:, 0:D], rcp[:])
                        row0 = qc * 512 + qt_i * 128
                        nc.sync.dma_start(
                            out=out_ext[row0:row0 + 128, :], in_=ot[:]
                        )

                # chunk 0: ST stream with chain qt0 lagging 2 kbp behind
                for kbp in range(NKB // 2):
                    st_step(0, kbp)
                    if kbp >= 2:
                        pv_part2(0, 0, (kbp - 2) * 2, (kbp - 2) * 2 + 2)
                pv_part2(0, 0, NKB - 4, NKB)
                # chunk 1 STs interleaved with chains qt1-3 of chunk 0
                # (flat list of 192 MMs, 6 per step, chains stay in order)
                blocks = []
                for qt_i in range(1, NQT):
                    for kb in range(NKB):
                        blocks.append((qt_i, kb))
                bi = 0
                for step in range(NKB // 2):
                    st_step(1, step)
                    take = 6 if step < 31 else len(blocks) - bi
                    for _ in range(take):
                        qt_i, kb = blocks[bi]
                        bi += 1
                        pv_part2(0, qt_i, kb, kb + 1)
                # chunk 1 PV chains
                for qt_i in range(NQT):
                    pv_part2(1, qt_i, 0, NKB)
            elif variant == "stag":
                # qc0: plain ST phase. Then steps interleaving one ST(qc1)
                # pair with 8 contiguous PV(qc0) chain matmuls (chains stay
                # qt-major / single-bank). Finally PV(qc1) qt-major.
                def pv_chain_part(qc, qt_i, kb0, kb1):
                    if kb0 == 0:
                        chains[(qc, qt_i)] = pvp.tile(
                            [128, D + 1], f32, tag="pv", name="pv"
                        )
                    pv = chains[(qc, qt_i)]
                    for kb in range(kb0, kb1):
                        col0 = (kb % 2) * 512 + qt_i * 128
                        nc.tensor.matmul(
                            pv[:],
                            pts[qc][kb // 2][:, col0:col0 + 128],
                            vo[kb // 4][:, kb % 4, :],
                            start=(kb == 0),
                            stop=(kb == NKB - 1),
                        )
                    if kb1 == NKB:
                        rcp = outp.tile([128, 1], f32, tag="rcp", name="rcp")
                        nc.vector.reciprocal(rcp[:], pv[:, D:D + 1])
                        ot = outp.tile([128, D], f32, tag="ot", name="ot")
                        nc.vector.tensor_scalar_mul(ot[:], pv[:, 0:D], rcp[:])
                        row0 = qc * 512 + qt_i * 128
                        nc.sync.dma_start(
                            out=out_ext[row0:row0 + 128, :], in_=ot[:]
                        )

                for kbp in range(NKB // 2):
                    st_step(0, kbp)
                for step in range(32):
                    st_step(1, step)
                    qt_i, seg = divmod(step, 8)
                    pv_chain_part(0, qt_i, seg * 8, seg * 8 + 8)
                for qt_i in range(NQT):
                    pv_chain_part(1, qt_i, 0, NKB)
            elif variant == "seq4":
                # per-chunk: all ST, then kb-major PV (all 4 chains advance
                # together so pt slots release early for the next chunk)
                for qc in range(NQC):
                    for kbp in range(NKB // 2):
                        st_step(qc, kbp)
                    for kb in range(NKB):
                        pv_kb(qc, kb)
            else:
                # sequential: all ST of a chunk, then qt-major PV chains
                for qc in range(NQC):
                    for kbp in range(NKB // 2):
                        st_step(qc, kbp)
                    for qt_i in range(NQT):
                        pv = pvp.tile([128, D + 1], f32, tag="pv", name="pv")
                        for kb in range(NKB):
                            col0 = (kb % 2) * 512 + qt_i * 128
                            nc.tensor.matmul(
                                pv[:],
                                pts[qc][kb // 2][:, col0:col0 + 128],
                                vo[kb // 4][:, kb % 4, :],
                                start=(kb == 0),
                                stop=(kb == NKB - 1),
                            )
                        rcp = outp.tile([128, 1], f32, tag="rcp", name="rcp")
                        nc.vector.reciprocal(rcp[:], pv[:, D:D + 1])
                        ot = outp.tile([128, D], f32, tag="ot", name="ot")
                        nc.vector.tensor_scalar_mul(ot[:], pv[:, 0:D], rcp[:])
                        row0 = qc * 512 + qt_i * 128
                        nc.sync.dma_start(
                            out=out_ext[row0:row0 + 128, :], in_=ot[:]
                        )


def _get_nc(repeat=1, variant=None):
    if variant is None:
        variant = DEFAULT_VARIANT
    key = f"nc{repeat}-{variant}"
    if key not in _CACHE:
        if variant.startswith("vb"):
            # vb[_L<lag>][_<order><batch>][_w<nwarm>]
            lag, order, batch, nwarm = 3, "hj", 2, 12
            for part in variant.split("_")[1:]:
                if part.startswith("L"):
                    lag = int(part[1:])
                elif part.startswith("w"):
                    nwarm = int(part[1:])
                elif part[:2] in ("jh", "hj"):
                    order, batch = part[:2], int(part[2:])
            _CACHE[key] = _build_vb(repeat, lag, order, batch, nwarm)
        else:
            _CACHE[key] = _build(repeat, variant)
    return _CACHE[key]


def run(inputs, trace=False):
    """Run on 8 cores; returns (full_output, BassKernelResults)."""
    from concourse.bass_utils import run_bass_kernel_spmd

    Q = np.ascontiguousarray(np.asarray(inputs["Q"], dtype=np.float32))
    K = np.ascontiguousarray(np.asarray(inputs["K"], dtype=np.float32))
    V = np.ascontiguousarray(np.asarray(inputs["V"], dtype=np.float32))

    nc = _get_nc()
    in_maps = [
        {"Q": Q[i * SHARD:(i + 1) * SHARD], "K": K, "V": V}
        for i in range(N_CORES)
    ]
    res = run_bass_kernel_spmd(
        nc, in_maps, core_ids=list(range(N_CORES)), trace=trace
    )
    out = np.concatenate([res.results[i]["out"] for i in range(N_CORES)], axis=0)
    return out, res


def kernel(**inputs) -> np.ndarray:
    import time

    last_err = None
    for attempt in range(3):
        try:
            out, _ = run(inputs, trace=False)
            return out
        except Exception as e:  # transient axon/device wedge - retry
            last_err = e
            time.sleep(15 * (attempt + 1))
    raise last_err

